# revision 26
# baseline (speedup 1.0000x reference)
"""AttentiveGraphConvolution (GAT-style layer) on 8 trn2 NeuronCores.

Math (reference):
    h   = x @ W                       [N, D]
    a_s = h @ attn_self               [N, 1]
    a_n = h @ attn_neigh              [N, 1]
    e   = leaky_relu(a_s + a_n.T, 0.2)
    e   = e + NEG_INF * (1 - adj)
    out = relu(softmax(e, -1) @ h)

Reformulation (exact up to rounding; leaky alpha = 0.2):
    t_ij = adj_ij * u2_i * v2_j * max(w_i * w2_j, 1)
    with u2 = e^{0.2 a_s}, v2 = e^{0.2 a_n}, w = e^{0.8 a_s}, w2 = e^{0.8 a_n};
    u2_i cancels in the softmax ratio, so with
        m_j   = e^{-0.8 a_n_j}
        q_ji  = adjT_ji * max(w_i, m_j)          (one DVE op per tile)
        h2_j  = e^{a_n_j} * h_j                  (folded into the stationary)
        ean_j = e^{a_n_j}
    out_i = relu( (sum_j q_ji h2_j) / (sum_j q_ji ean_j) ).

ZERO-COLLECTIVE design: the NRT collective rendezvous costs 40-70 us of
core-launch stagger plus ~20 us fixed cost per collective, so instead of
AllGathering h, EVERY core computes the full h = x @ W itself from a
replicated fp8 copy of x (13.6 us of PE + 4 MB of DMA).  No collectives,
no barrier: each core's span is pure local compute, immune to stagger.

Hot loop per adj chunk [128 j, 1024 i]:
    DVE  scalar_tensor_tensor: q = (wb max m_j) * adjT      (bf16)
    PE   2x matmul [128,512]:  outT += h2_chunk.T @ q
    PE   2x matmul [1,512]:    rs   += ean_chunk.T @ q
plus a pipelined PE transpose + ACT scale producing each h2 stationary.

Sharding: output rows across 8 cores; adj arrives pre-transposed bf16 with
a 4-row interleave for 8 KB DMA descriptors; the full adj row-slab streams
into SBUF through a 12-deep ring ahead of the DVE.
"""

import numpy as np

N = 8192
DIN = 512
DOUT = 128
NCORES = 8
S = N // NCORES     # 1024 output rows per core
GP = 4              # adjacency rows per partition per DMA (descriptor size)


def _emit(nc, tc, ctx, n, s, din, dout):
    from concourse import masks, mybir

    f32 = mybir.dt.float32
    bf16 = mybir.dt.bfloat16
    AF = mybir.ActivationFunctionType
    ALU = mybir.AluOpType

    P = 128
    jc_n = n // P       # j chunks over all nodes (64)
    sc_n = s // P       # chunks in the local row slab (8)
    kc_n = din // P     # contraction chunks for x @ W (4)
    g_n = jc_n // GP    # adj super-chunks (GP j-chunks per DMA) (16)
    hb_n = 16           # hT blocks of 512 columns
    xp_n = 4            # column pieces per xf chunk DMA
    grp = 4             # m/ean column groups
    gj = jc_n // grp    # chunks per group (16)

    adjt = nc.dram_tensor("adjt", [n, s], bf16, kind="ExternalInput")
    xf = nc.dram_tensor("xf", [din, n], bf16, kind="ExternalInput")
    wf = nc.dram_tensor("wf", [din, dout], bf16, kind="ExternalInput")
    xl = nc.dram_tensor("xl", [din, s], bf16, kind="ExternalInput")
    wt = nc.dram_tensor("wt", [dout, din], bf16, kind="ExternalInput")
    att = nc.dram_tensor("att", [dout, 2], bf16, kind="ExternalInput")
    out = nc.dram_tensor("out", [s, dout], f32, kind="ExternalOutput")

    const_pool = ctx.enter_context(tc.tile_pool(name="const", bufs=1))
    ph1_pool = ctx.enter_context(tc.tile_pool(name="ph1", bufs=1))
    tp_psum = ctx.enter_context(tc.tile_pool(name="tp_psum", bufs=2, space="PSUM"))
    acc_psum = ctx.enter_context(tc.tile_pool(name="acc_psum", bufs=1, space="PSUM"))
    dram_pool = ctx.enter_context(tc.tile_pool(name="dram", bufs=1, space="DRAM"))
    adj_pool = ctx.enter_context(tc.tile_pool(name="adj", bufs=9))
    q_pool = ctx.enter_context(tc.tile_pool(name="q", bufs=8))
    stat_pool = ctx.enter_context(tc.tile_pool(name="stat", bufs=8))
    hnat_pool = ctx.enter_context(tc.tile_pool(name="hnat", bufs=4))
    fin_pool = ctx.enter_context(tc.tile_pool(name="fin", bufs=2))

    # ---- Phase 0: input DMAs (small first, then xf pieces, then adj) -----
    xl_sb = []
    for k in range(kc_n):
        t = ph1_pool.tile([P, s], bf16, name="xl_sb", tag=f"xl{k}")
        nc.sync.dma_start(t[:], xl[k * P:(k + 1) * P, :])
        xl_sb.append(t)
    wt_sb = ph1_pool.tile([P, din], bf16, name="wt_sb")
    nc.sync.dma_start(wt_sb[:], wt[:])
    att_sb = const_pool.tile([P, 2], bf16, name="att_sb")
    nc.sync.dma_start(att_sb[:], att[:])
    wf_sb = []
    for k in range(kc_n):
        t = ph1_pool.tile([P, dout], bf16, name="wf_sb", tag=f"wf{k}")
        nc.sync.dma_start(t[:], wf[k * P:(k + 1) * P, :])
        wf_sb.append(t)
    # xf pieces: xfp[k][p] covers columns [p*2048, (p+1)*2048)
    pw = n // xp_n
    xfp = [[None] * xp_n for _ in range(kc_n)]
    for p in range(xp_n):
        for k in range(kc_n):
            t = ph1_pool.tile([P, pw], bf16, name="xfp", tag=f"xfp{k}_{p}")
            nc.sync.dma_start(t[:], xf[k * P:(k + 1) * P, p * pw:(p + 1) * pw])
            xfp[k][p] = t

    # first 10 adj super-chunks up front; the rest interleave with the main
    # loop emission so their ring-slot waits never block later DMA triggers
    adj_t = []

    def emit_adj(g):
        at = adj_pool.tile([P, GP * s], bf16, name="adj_t")
        nc.sync.dma_start(
            at[:],
            adjt[g * GP * P:(g + 1) * GP * P, :].rearrange(
                "(p r) i -> p (r i)", r=GP),
        )
        adj_t.append(at)

    for g in range(9):
        emit_adj(g)

    ident = const_pool.tile([P, P], f32, name="ident")
    masks.make_identity(nc, ident[:])
    identb = const_pool.tile([P, P], bf16, name="identb")
    nc.scalar.activation(identb[:], ident[:], AF.Copy)
    ones_bf = const_pool.tile([1, P], bf16, name="ones_bf")
    nc.gpsimd.memset(ones_bf[:], 1.0)

    # ---- Phase 1: local a_s -> wb = e^{0.8 a_s_i} broadcast (bf16) -------
    av2_ps = tp_psum.tile([2, din], f32, name="av2_ps", tag="tp")
    nc.tensor.matmul(av2_ps[:], att_sb[:], wt_sb[:], start=True, stop=True)
    av2_sb = ph1_pool.tile([2, din], bf16, name="av2_sb")
    nc.scalar.activation(av2_sb[:], av2_ps[:], AF.Copy)
    av2T_sb = []
    for k in range(kc_n):
        avT_ps = tp_psum.tile([P, 2], bf16, name="avT_ps", tag="tr")
        nc.tensor.matmul(
            avT_ps[:], av2_sb[:, k * P:(k + 1) * P], identb[:2, :2],
            is_transpose=True, start=True, stop=True,
        )
        a2t = ph1_pool.tile([P, 2], bf16, name="av2T_sb", tag=f"av2T{k}")
        nc.scalar.activation(a2t[:], avT_ps[:], AF.Copy)
        av2T_sb.append(a2t)
    as_sb = ph1_pool.tile([1, s], f32, name="as_sb")
    for b in range(s // 512):
        avl_ps = tp_psum.tile([2, 512], f32, name="avl_ps", tag="tp")
        for k in range(kc_n):
            nc.tensor.matmul(
                avl_ps[:], av2T_sb[k][:], xl_sb[k][:, b * 512:(b + 1) * 512],
                start=(k == 0), stop=(k == kc_n - 1),
            )
        nc.scalar.activation(as_sb[:, b * 512:(b + 1) * 512], avl_ps[0:1, :],
                             AF.Copy)
    wrow_bf = ph1_pool.tile([1, s], bf16, name="wrow_bf")
    nc.scalar.activation(wrow_bf[:], as_sb[:], AF.Exp, scale=0.8)
    wb_sb = const_pool.tile([P, s], bf16, name="wb_sb")
    for b in range(s // 512):
        wb_ps = tp_psum.tile([P, 512], f32, name="wb_ps", tag="tp")
        nc.tensor.matmul(wb_ps[:], ones_bf[:], wrow_bf[:, b * 512:(b + 1) * 512],
                         start=True, stop=True)
        nc.scalar.activation(wb_sb[:, b * 512:(b + 1) * 512], wb_ps[:], AF.Copy)

    # ---- Phase 2: full hT = (x W).T in 512-col blocks; a_n per block -----
    # a_n row segments go through DRAM per column group; the read-back +
    # one PE transpose yields [node-partition, chunk] layout without
    # holding PSUM banks across the phase.
    hT_blk = []
    an_dram_g = [dram_pool.tile([4, 512], f32, name=f"an_dram{g}", tag=f"and{g}")
                 for g in range(grp)]
    for b in range(hb_n):
        hT_ps = tp_psum.tile([P, 512], f32, name="hT_ps", tag="tp")
        for k in range(kc_n):
            nc.tensor.matmul(
                hT_ps[:],
                wf_sb[k][:],
                xfp[k][b // xp_n][:, (b % xp_n) * 512:(b % xp_n + 1) * 512],
                start=(k == 0), stop=(k == kc_n - 1),
            )
        hb = ph1_pool.tile([P, 512], bf16, name="hT_blk", tag=f"hT{b}")
        nc.scalar.activation(hb[:], hT_ps[:], AF.Copy)
        hT_blk.append(hb)
        # a_n for this block's 4 chunks: av = attT @ hT
        avf_ps = tp_psum.tile([2, 512], f32, name="avf_ps", tag="tp")
        nc.tensor.matmul(avf_ps[:], att_sb[:], hb[:], start=True, stop=True)
        avf_sb = fin_pool.tile([2, 512], f32, name="avf_sb")
        nc.scalar.activation(avf_sb[:], avf_ps[:], AF.Copy)
        nc.sync.dma_start(an_dram_g[b // 4][b % 4:b % 4 + 1, :], avf_sb[1:2, :])

    # m = e^{-0.8 a_n} (bf16 STT scalar), ean bf16 (rs stationary),
    # eanf f32 (ACT scale for the h2 stationaries) -- per column group
    m_g, ean_g, eanf_g = [], [], []
    for g in range(grp):
        anr = ph1_pool.tile([gj, P], f32, name="anr_g", tag=f"anr{g}")
        nc.sync.dma_start(anr[:], an_dram_g[g][:].rearrange(
            "b (t p) -> (b t) p", p=P))
        anT_ps = tp_psum.tile([P, gj], f32, name="anT_ps", tag="tr")
        nc.tensor.matmul(anT_ps[:], anr[:], ident[:gj, :gj],
                         is_transpose=True, start=True, stop=True)
        mg = ph1_pool.tile([P, gj], bf16, name="m_g", tag=f"m{g}")
        nc.scalar.activation(mg[:], anT_ps[:], AF.Exp, scale=-0.8)
        m_g.append(mg)
        eg = ph1_pool.tile([P, gj], bf16, name="ean_g", tag=f"ean{g}")
        nc.scalar.activation(eg[:], anT_ps[:], AF.Exp, scale=1.0)
        ean_g.append(eg)
        ef = ph1_pool.tile([P, gj], f32, name="eanf_g", tag=f"eanf{g}")
        nc.scalar.activation(ef[:], anT_ps[:], AF.Exp, scale=1.0)
        eanf_g.append(ef)

    # ---- Phase 3: main loop; h2 stationaries pipelined LEAD chunks ahead -
    # Stationary transposes run on the DMA crossbar (dma_start_transpose),
    # keeping the PE free for the four accumulation matmuls per chunk; the
    # elementwise mask op runs on GpSimd for every 4th chunk to unload DVE.
    nb = 512
    ib_n = s // nb
    LEAD = 6
    mm_ps = [acc_psum.tile([P, nb], f32, name=f"mm_ps{b}") for b in range(ib_n)]
    rs_ps = [acc_psum.tile([1, nb], f32, name=f"rs_ps{b}") for b in range(ib_n)]
    stats = {}

    def emit_stat(j):
        hnat = hnat_pool.tile([P, P], bf16, name="hnat")
        nc.sync.dma_start_transpose(
            hnat[:], hT_blk[j // 4][:, (j % 4) * P:(j % 4 + 1) * P])
        st = stat_pool.tile([P, dout], bf16, name="stat")
        nc.scalar.activation(st[:], hnat[:], AF.Copy,
                             scale=eanf_g[j // gj][:, j % gj:j % gj + 1])
        stats[j] = st

    def emit_body(j):
        q_t = q_pool.tile([P, s], bf16, name="q_t")
        nc.vector.scalar_tensor_tensor(
            q_t[:], wb_sb[:], m_g[j // gj][:, j % gj:j % gj + 1],
            adj_t[j // GP][:, (j % GP) * s:(j % GP + 1) * s],
            op0=ALU.max, op1=ALU.mult,
        )
        st = stats.pop(j)
        for b in range(ib_n):
            nc.tensor.matmul(
                mm_ps[b][:], st[:], q_t[:, b * nb:(b + 1) * nb],
                start=(j == 0), stop=(j == jc_n - 1),
            )
        for b in range(ib_n):
            nc.tensor.matmul(
                rs_ps[b][:], ean_g[j // gj][:, j % gj:j % gj + 1],
                q_t[:, b * nb:(b + 1) * nb],
                start=(j == 0), stop=(j == jc_n - 1),
            )

    for idx in range(jc_n + LEAD):
        if idx < jc_n:
            if idx % 4 == 0 and 9 + idx // 4 < g_n:
                emit_adj(9 + idx // 4)
            emit_stat(idx)
        if idx >= LEAD:
            emit_body(idx - LEAD)

    # ---- Phase 4: normalize, relu, transpose out -------------------------
    rs_sb = ph1_pool.tile([1, s], f32, name="rs_sb")
    for b in range(ib_n):
        nc.scalar.activation(rs_sb[:, b * nb:(b + 1) * nb], rs_ps[b][:], AF.Copy)
    rs_dram = dram_pool.tile([sc_n, P], f32, name="rs_dram")
    nc.sync.dma_start(rs_dram[:].rearrange("k p -> (k p)")[None, :], rs_sb[0:1, :])
    rs_raw = ph1_pool.tile([sc_n, P], f32, name="rs_raw")
    nc.sync.dma_start(rs_raw[:], rs_dram[:])
    rsT_ps = tp_psum.tile([P, sc_n], f32, name="rsT_ps", tag="tp")
    nc.tensor.matmul(rsT_ps[:], rs_raw[:], ident[:sc_n, :sc_n],
                     is_transpose=True, start=True, stop=True)
    rrT_sb = ph1_pool.tile([P, sc_n], f32, name="rrT_sb")
    nc.vector.reciprocal(rrT_sb[:], rsT_ps[:])

    mo_sb = ph1_pool.tile([P, s], f32, name="mo_sb")
    for b in range(ib_n):
        nc.scalar.activation(mo_sb[:, b * nb:(b + 1) * nb], mm_ps[b][:], AF.Copy)
    for c in range(sc_n):
        ot_ps = tp_psum.tile([P, P], f32, name="ot_ps", tag="tp")
        nc.tensor.matmul(
            ot_ps[:], mo_sb[:, c * P:(c + 1) * P], ident[:],
            is_transpose=True, start=True, stop=True,
        )
        oc_sb = fin_pool.tile([P, dout], f32, name="oc_sb")
        nc.scalar.activation(oc_sb[:], ot_ps[:], AF.Relu,
                             scale=rrT_sb[:, c:c + 1])
        nc.sync.dma_start(out[c * P:(c + 1) * P, :], oc_sb[:])


def build_nc(n=N, s=S, din=DIN, dout=DOUT):
    from contextlib import ExitStack

    import concourse.bacc as bacc
    import concourse.tile as tile

    nc = bacc.Bacc(
        "TRN2",
        target_bir_lowering=False,
        debug=False,
        num_devices=NCORES,
    )
    with tile.TileContext(nc) as tc, ExitStack() as ctx:
        _emit(nc, tc, ctx, n, s, din, dout)
    nc.compile()
    return nc


def prep_adjt(adj_slab):
    """[s, n] adj row-slab -> transposed [n, s] bf16 with GP-row interleave."""
    import ml_dtypes

    adjt = adj_slab.T  # [n, s]
    n, s = adjt.shape
    P = 128
    g = n // (GP * P)
    adjt = adjt.reshape(g, GP, P, s).transpose(0, 2, 1, 3).reshape(n, s)
    return np.ascontiguousarray(adjt.astype(ml_dtypes.bfloat16))


def make_in_maps(x, adj, W, attn_self, attn_neigh, s=S):
    import ml_dtypes

    bf = ml_dtypes.bfloat16
    att = np.concatenate([attn_self, attn_neigh], axis=1).astype(bf)
    xfull = np.ascontiguousarray(x.T.astype(bf))
    wfull = np.ascontiguousarray(W.astype(bf))
    wtt = np.ascontiguousarray(W.T.astype(bf))
    in_maps = []
    for c in range(NCORES):
        sl = slice(c * s, (c + 1) * s)
        in_maps.append({
            "adjt": prep_adjt(adj[sl, :]),
            "xf": xfull,
            "wf": wfull,
            "xl": np.ascontiguousarray(x[sl, :].T.astype(bf)),
            "wt": wtt,
            "att": att,
        })
    return in_maps


def kernel(x, adj, W, attn_self, attn_neigh):
    from concourse.bass_utils import run_bass_kernel_spmd

    x = np.asarray(x, dtype=np.float32)
    adj = np.asarray(adj, dtype=np.float32)
    W = np.asarray(W, dtype=np.float32)
    attn_self = np.asarray(attn_self, dtype=np.float32)
    attn_neigh = np.asarray(attn_neigh, dtype=np.float32)

    nc = build_nc()
    in_maps = make_in_maps(x, adj, W, attn_self, attn_neigh)
    res = run_bass_kernel_spmd(nc, in_maps, list(range(NCORES)))
    return np.concatenate([res.results[c]["out"] for c in range(NCORES)], axis=0)
